# revision 8
# baseline (speedup 1.0000x reference)
"""ConvTimeAware-LSTM Trainium2 kernel.

Data-parallel over batch: 8 NeuronCores, one batch element each.
Per core, per timestep t (B=1, T=16, Cin=3, Hd=64, H=W=64, k=3 SAME):
    i = f = sigmoid(conv(cat(x,h), Wi) + bi)
    c_S = conv(c, Wcs) + bcs
    c_star = c - c_S * (1 + tanh(tm))          [1+tanh(z) = 2*sigmoid(2z)]
    c_tilde = tanh(conv(cat(x,h), Wct) + bct)
    c_new = i * (c_star + c_tilde)
    o = sigmoid(conv(cat(x,h,tm), Wo) + bo)
    h_new = o * tanh(c_new)

Convs are computed as 9 shifted fp32r matmuls accumulating in PSUM over a
zero-padded [C, 66, 66] SBUF image; channel layout [h(0:64), x(64:67), tm(67)].
The (1+tanh(tm)) per-pixel factor becomes sigma = sigmoid(2*tm) computed once
for all T, with the factor 2 folded into Wcs/bcs host-side; sigma rows are
broadcast across the 64 channel partitions by a 0-stride DMA from DRAM.
"""
import numpy as np
from contextlib import ExitStack

import concourse.bass as bass
import concourse.bacc as bacc
import concourse.tile as tile
import concourse.mybir as mybir
from concourse.bass_utils import run_bass_kernel_spmd

dt = mybir.dt
AF = mybir.ActivationFunctionType
ALU = mybir.AluOpType

T, CIN, HD, H, W = 16, 3, 64, 64, 64
PW = W + 2                    # padded width
PADN = PW * PW                # padded image size per channel
K_XH = HD + CIN + 1           # 68: h, x, tm partitions
NCH, RPC = 8, 8               # chunks per image, rows per chunk
CH = RPC * W                  # 512 free elements per chunk
NCORES = 8

_cache = {}


def _build(rep: int = 1, steps: int = T):
    nc = bacc.Bacc("TRN2", target_bir_lowering=False, debug=False,
                   enable_asserts=False, num_devices=NCORES)

    x_d = nc.dram_tensor("x", [T, CIN, H, W], dt.float32, kind="ExternalInput").ap()
    tm_d = nc.dram_tensor("tm", [T, H, W], dt.float32, kind="ExternalInput").ap()
    w_ict_d = nc.dram_tensor("w_ict", [K_XH, 9 * 128], dt.float32, kind="ExternalInput").ap()
    w_o_d = nc.dram_tensor("w_o", [K_XH, 9 * 64], dt.float32, kind="ExternalInput").ap()
    w_cs_d = nc.dram_tensor("w_cs", [HD, 9 * 64], dt.float32, kind="ExternalInput").ap()
    b_i_d = nc.dram_tensor("b_i", [HD, 1], dt.float32, kind="ExternalInput").ap()
    b_ct_d = nc.dram_tensor("b_ct", [HD, 1], dt.float32, kind="ExternalInput").ap()
    b_o_d = nc.dram_tensor("b_o", [HD, 1], dt.float32, kind="ExternalInput").ap()
    b_cs_d = nc.dram_tensor("b_cs", [HD, 1], dt.float32, kind="ExternalInput").ap()
    tmsig_d = nc.dram_tensor("tmsig", [128, T * H * W // 128], dt.float32,
                             kind="ExternalInput").ap()

    out_hs = nc.dram_tensor("out_hs", [T, HD, H, W], dt.float32, kind="ExternalOutput").ap()
    out_c = nc.dram_tensor("out_c", [HD, H, W], dt.float32, kind="ExternalOutput").ap()

    sig_dram = nc.dram_tensor("sig_scratch", [T, H * W], dt.float32, kind="Internal").ap()

    with tile.TileContext(nc) as tc, ExitStack() as ctx:
        consts = ctx.enter_context(tc.tile_pool(name="consts", bufs=1))
        state = ctx.enter_context(tc.tile_pool(name="state", bufs=1))
        sigp = ctx.enter_context(tc.tile_pool(name="sigp", bufs=2))
        work = ctx.enter_context(tc.tile_pool(name="work", bufs=3))
        ps = ctx.enter_context(tc.tile_pool(name="ps", bufs=2, space="PSUM"))

        # ---- constants ----
        w_ict = consts.tile([K_XH, 9 * 128], dt.float32r)
        nc.gpsimd.dma_start(w_ict[:], w_ict_d)
        w_o = consts.tile([K_XH, 9 * 64], dt.float32r)
        nc.gpsimd.dma_start(w_o[:], w_o_d)
        w_cs = consts.tile([HD, 9 * 64], dt.float32r)
        nc.gpsimd.dma_start(w_cs[:], w_cs_d)
        b_i = consts.tile([HD, 1], dt.float32)
        nc.sync.dma_start(b_i[:], b_i_d)
        b_ct = consts.tile([HD, 1], dt.float32)
        nc.sync.dma_start(b_ct[:], b_ct_d)
        b_o = consts.tile([HD, 1], dt.float32)
        nc.sync.dma_start(b_o[:], b_o_d)
        b_cs = consts.tile([HD, 1], dt.float32)
        nc.sync.dma_start(b_cs[:], b_cs_d)

        # ---- sigma = sigmoid(2*tm) for all T, staged through DRAM for the
        # per-timestep 0-stride partition broadcast ----
        tm_sb = consts.tile([128, T * H * W // 128], dt.float32)
        nc.sync.dma_start(tm_sb[:], tmsig_d)
        sig_sb = consts.tile([128, T * H * W // 128], dt.float32)
        nc.scalar.activation(sig_sb[:], tm_sb[:], AF.Sigmoid, scale=2.0)
        nc.sync.dma_start(sig_dram.rearrange("t n -> (t n)").rearrange("(p n) -> p n", p=128),
                          sig_sb[:])

        # ---- state (ping-pong padded images) ----
        xht0 = state.tile([K_XH, PADN], dt.float32r)
        xht1 = state.tile([K_XH, PADN], dt.float32r)
        cpad0 = state.tile([HD, PADN], dt.float32r)
        cpad1 = state.tile([HD, PADN], dt.float32r)
        xht = [xht0, xht1]
        cpad = [cpad0, cpad1]
        # fp32r tiles cannot be memset directly (walrus ISA check); zero them
        # by copy from a zeroed fp32 tile (tensor_copy rounds to fp32r).
        zsrc = state.tile([K_XH, PADN], dt.float32)
        nc.vector.memset(zsrc[:], 0.0)
        for tl in (xht0, xht1, cpad0, cpad1):
            nc.vector.tensor_copy(tl[:], zsrc[0:tl.shape[0], :])

        def interior(tl, q, nparts=HD):
            """Chunk q's interior window [nparts, RPC rows, W cols]."""
            v = tl[:].rearrange("p (r c) -> p r c", c=PW)
            return v[0:nparts, RPC * q + 1: RPC * q + 1 + RPC, 1: 1 + W]

        def window(tl, nparts, q, dy, dx):
            """rhs view for tap (dy,dx) of chunk q."""
            v = tl[:].rearrange("p (r c) -> p r c", c=PW)
            return v[0:nparts, RPC * q + dy: RPC * q + dy + RPC, dx: dx + W]

        for r in range(rep):
            # reset recurrent state read at t=0
            nc.vector.tensor_copy(xht0[0:HD, :], zsrc[0:HD, :])
            nc.vector.tensor_copy(cpad0[:], zsrc[0:HD, :])

            for t in range(steps):
                cur, nxt = xht[t % 2], xht[(t + 1) % 2]
                ccur, cnxt = cpad[t % 2], cpad[(t + 1) % 2]

                # stage x_{t}/tm_{t} into cur (they are consumed by step t)
                nc.gpsimd.dma_start(cur[:].rearrange("p (r c) -> p r c", c=PW)
                                    [HD:HD + CIN, 1:1 + H, 1:1 + W],
                                    x_d[t])
                nc.gpsimd.dma_start(cur[:].rearrange("p (r c) -> p r c", c=PW)
                                    [HD + CIN:K_XH, 1:1 + H, 1:1 + W],
                                    tm_d[t:t + 1])

                # per-timestep sigma row broadcast across 64 partitions
                sig_bc = sigp.tile([HD, H * W], dt.float32, name="sig_bc")
                nc.sync.dma_start(sig_bc[:], sig_dram[t:t + 1].broadcast_to((HD, H * W)))

                for q in range(NCH):
                    p_ict = ps.tile([128, CH], dt.float32, name="p_ict")
                    p_o = ps.tile([HD, CH], dt.float32, name="p_o")
                    p_cs = ps.tile([HD, CH], dt.float32, name="p_cs")
                    for dy in range(3):
                        for dx in range(3):
                            tap = dy * 3 + dx
                            rv = window(cur, K_XH, q, dy, dx)
                            nc.tensor.matmul(p_ict[:], w_ict[:, tap * 128:(tap + 1) * 128],
                                             rv, start=(tap == 0), stop=(tap == 8))
                            nc.tensor.matmul(p_o[:], w_o[:, tap * 64:(tap + 1) * 64],
                                             rv, start=(tap == 0), stop=(tap == 8))
                            cv = window(ccur, HD, q, dy, dx)
                            nc.tensor.matmul(p_cs[:], w_cs[:, tap * 64:(tap + 1) * 64],
                                             cv, start=(tap == 0), stop=(tap == 8))

                    i_sb = work.tile([HD, CH], dt.float32, name="i_sb")
                    nc.scalar.activation(i_sb[:], p_ict[0:HD, :], AF.Sigmoid, bias=b_i[:])
                    ct_sb = work.tile([HD, CH], dt.float32, name="ct_sb")
                    nc.scalar.activation(ct_sb[:], p_ict[HD:128, :], AF.Tanh, bias=b_ct[:])
                    o_sb = work.tile([HD, CH], dt.float32, name="o_sb")
                    nc.scalar.activation(o_sb[:], p_o[:], AF.Sigmoid, bias=b_o[:])

                    # m1 = (c_S_conv + bcs') * sigma   (the "2x" lives in w_cs/b_cs)
                    m1 = work.tile([HD, CH], dt.float32, name="m1")
                    nc.vector.scalar_tensor_tensor(m1[:], p_cs[:], b_cs[:],
                                                   sig_bc[:, q * CH:(q + 1) * CH],
                                                   op0=ALU.add, op1=ALU.mult)
                    c2 = work.tile([HD, CH], dt.float32, name="c2")
                    nc.vector.tensor_tensor(c2[:], interior(ccur, q), m1[:], op=ALU.subtract)
                    c3 = work.tile([HD, CH], dt.float32, name="c3")
                    nc.vector.tensor_tensor(c3[:], c2[:], ct_sb[:], op=ALU.add)
                    nc.vector.tensor_tensor(interior(cnxt, q), i_sb[:], c3[:], op=ALU.mult)

                    tc_sb = work.tile([HD, CH], dt.float32, name="tc_sb")
                    nc.scalar.activation(tc_sb[:], interior(cnxt, q), AF.Tanh)
                    nc.vector.tensor_tensor(interior(nxt, q), o_sb[:], tc_sb[:], op=ALU.mult)

                # h_t -> DRAM (cast fp32r -> fp32 via gpsimd DMA)
                nc.gpsimd.dma_start(out_hs[t],
                                    nxt[:].rearrange("p (r c) -> p r c", c=PW)
                                    [0:HD, 1:1 + H, 1:1 + W])

            if r == rep - 1:
                nc.gpsimd.dma_start(out_c,
                                    cpad[steps % 2][:].rearrange("p (r c) -> p r c", c=PW)
                                    [0:HD, 1:1 + H, 1:1 + W])

    nc.compile()
    return nc


def _get_nc(rep: int = 1):
    if rep not in _cache:
        _cache[rep] = _build(rep)
    return _cache[rep]


def _prep_weights(Wi, bi, Wct, bct, Wo, bo, Wcs, bcs):
    """Host-side reshape into lhsT layouts. Channel partitions: h 0:64,
    x 64:67, tm 67. Factor 2 folded into Wcs/bcs (1+tanh = 2*sigmoid(2x))."""
    w_ict = np.zeros((K_XH, 9 * 128), np.float32)
    w_o = np.zeros((K_XH, 9 * 64), np.float32)
    w_cs = np.zeros((HD, 9 * 64), np.float32)
    for dy in range(3):
        for dx in range(3):
            tap = dy * 3 + dx
            s = tap * 128
            w_ict[0:HD, s:s + 64] = Wi[:, CIN:CIN + HD, dy, dx].T
            w_ict[HD:HD + CIN, s:s + 64] = Wi[:, 0:CIN, dy, dx].T
            w_ict[0:HD, s + 64:s + 128] = Wct[:, CIN:CIN + HD, dy, dx].T
            w_ict[HD:HD + CIN, s + 64:s + 128] = Wct[:, 0:CIN, dy, dx].T
            so = tap * 64
            w_o[0:HD, so:so + 64] = Wo[:, CIN:CIN + HD, dy, dx].T
            w_o[HD:HD + CIN, so:so + 64] = Wo[:, 0:CIN, dy, dx].T
            w_o[HD + CIN, so:so + 64] = Wo[:, CIN + HD, dy, dx]
            w_cs[:, so:so + 64] = 2.0 * Wcs[:, :, dy, dx].T
    return {
        "w_ict": w_ict, "w_o": w_o, "w_cs": w_cs,
        "b_i": bi.reshape(HD, 1).astype(np.float32),
        "b_ct": bct.reshape(HD, 1).astype(np.float32),
        "b_o": bo.reshape(HD, 1).astype(np.float32),
        "b_cs": (2.0 * bcs).reshape(HD, 1).astype(np.float32),
    }


def make_in_maps(input_tensor, time_tensor, Wi, bi, Wct, bct, Wo, bo, Wcs, bcs):
    wts = _prep_weights(np.asarray(Wi), np.asarray(bi), np.asarray(Wct),
                        np.asarray(bct), np.asarray(Wo), np.asarray(bo),
                        np.asarray(Wcs), np.asarray(bcs))
    input_tensor = np.ascontiguousarray(np.asarray(input_tensor), np.float32)
    time_tensor = np.ascontiguousarray(np.asarray(time_tensor), np.float32)
    in_maps = []
    for b in range(NCORES):
        tm_b = time_tensor[b, :, 0]                       # [T, H, W]
        in_maps.append({
            "x": np.ascontiguousarray(input_tensor[b]),
            "tm": np.ascontiguousarray(tm_b),
            "tmsig": np.ascontiguousarray(tm_b.reshape(128, T * H * W // 128)),
            **wts,
        })
    return in_maps


def kernel(input_tensor, time_tensor, Wi, bi, Wct, bct, Wo, bo, Wcs, bcs):
    in_maps = make_in_maps(input_tensor, time_tensor, Wi, bi, Wct, bct,
                           Wo, bo, Wcs, bcs)
    nc = _get_nc()
    br = run_bass_kernel_spmd(nc, in_maps, core_ids=list(range(NCORES)))
    hs = np.stack([br.results[b]["out_hs"] for b in range(NCORES)])
    c_last = np.stack([br.results[b]["out_c"] for b in range(NCORES)])
    h_last = np.ascontiguousarray(hs[:, -1])
    return hs, h_last, c_last
